# revision 3
# baseline (speedup 1.0000x reference)
"""HashEmbedder3D Trainium2 kernel v2 — dma_gather block scheme.

Tables are bf16-packed (one int32 lane per 2-feature row) and repacked into
per-level block tables with 256B slot stride (dma_gather requires stride %
256B == 0; elem_size is free):
  - L0-2: slot s = rows (s, s+1): a k-pair is one 8B gather, no extraction.
  - L3-7: slot s = rows [G*s, G*s+G] (G=2..32); pair extracted with a bitwise
    select tree over the G+1 lanes.
  - hash: slot s = rows [16s, 16s+16); single row via 4-round select tree.
Indices are computed on DVE, round-tripped through a DRAM scratch to reach the
16-partition-wrapped int16 layout dma_gather reads, and each gather moves
16-32K indices so the ~1us SWDGE setup amortizes away.
"""
import math
import sys

import numpy as np

sys.path.insert(0, "/opt/trn_rl_repo")

from concourse import bacc, bass, mybir
import concourse.tile as tile

N_LEVELS = 16
F = 2
LOG2_T = 19
T = 1 << LOG2_T
BASE, FINEST = 16, 512
B_GROWTH = float(np.exp((np.log(np.float32(FINEST)) - np.log(np.float32(BASE))) / np.float32(N_LEVELS - 1)))
RES = [math.floor(BASE * B_GROWTH**i) for i in range(N_LEVELS)]
SIZES = [(r + 1) ** 3 if r**3 < T else T for r in RES]
OFFS = np.concatenate([[0], np.cumsum(SIZES)]).tolist()
TOTAL_ROWS = OFFS[-1]
PRIMES = [1, 2654435761, 805459861]
N_POINTS = 1048576
N_CORES = 8
P = 128

DT = mybir.dt
AL = mybir.AluOpType

DENSE_LV = list(range(8))
HASH_LV = list(range(8, 16))
LEVEL_G = {0: 1, 1: 1, 2: 1, 3: 2, 4: 4, 5: 8, 6: 16, 7: 32}
for l in HASH_LV:
    LEVEL_G[l] = 16
N_SLOTS, ELEM = {}, {}
for l in range(N_LEVELS):
    G = LEVEL_G[l]
    if G == 1:
        N_SLOTS[l], ELEM[l] = SIZES[l], 2
    elif l in DENSE_LV:
        N_SLOTS[l], ELEM[l] = -(-SIZES[l] // G), G + 1
    else:
        N_SLOTS[l], ELEM[l] = T // 16, 16
SLOT_OFF = {}
_a = 0
for l in range(N_LEVELS):
    SLOT_OFF[l] = _a
    _a += N_SLOTS[l]
TOT_SLOTS = _a
assert all(N_SLOTS[l] <= 32768 for l in range(N_LEVELS))

CC0 = {}
_c = 0
for l in range(N_LEVELS):
    CC0[l] = _c
    _c += 4 if l in DENSE_LV else 8
NCC = _c  # 96
CCOFF = {}
_c = 0
for l in range(3, N_LEVELS):
    CCOFF[l] = _c
    _c += 4 if l in DENSE_LV else 8
NCOFF = _c  # 84


def _i32(v):
    return int(np.int32(np.uint32(v)))


MAX_GIDX = 1024


def dma_gather_chunked(eng, gd, gd_off, el, in_ap, wrp, col0, num_idxs):
    """Split a gather into <=MAX_GIDX-idx instructions (proven HW size)."""
    done = 0
    while done < num_idxs:
        n = min(MAX_GIDX, num_idxs - done)
        gview = bass.AP(gd.tensor, gd.offset + gd_off + (done // 128) * el,
                        [list(gd.ap[0]), [el, n // 128], [1, el]])
        dma_gather_raw(eng, gview, in_ap, wrp[:, col0 + done // 16 : col0 + (done + n) // 16], n, el)
        done += n


def dma_gather_raw(eng, out_ap, in_ap, idxs_ap, num_idxs, elem_size, elem_step=64):
    stride_bytes = elem_step * 4
    assert stride_bytes % 256 == 0
    _in_ap = eng.lower_ap_dma(in_ap, for_custom_bir_dma=True)
    _idxs_ap = eng.lower_ap(idxs_ap)
    _out_ap = eng.lower_ap(out_ap)
    return eng.add_instruction(
        mybir.InstDMAGatherAnt(
            name=eng.bass.get_next_instruction_name(),
            ins=[*_in_ap, _idxs_ap, eng.lower_val_access(eng.to_reg(num_idxs))],
            outs=[_out_ap],
            transpose=False,
            num_idxs=num_idxs,
            elem_size=elem_size,
            stride_bytes_256=stride_bytes // 256,
            gen_mode=0,
            single_packet=True,
            queue_num=0,
            sbuf_tokens_per_rank=0,
            sbuf_free_dim_per_rank=0,
            sbuf_free_dim_pad_per_rank=0,
            sbuf_byte_offset=0,
        )
    )


def apd(tap, off, dims):
    return bass.AP(tap.tensor, tap.offset + off, [list(d) for d in dims])


def build_kernel(slots_total, S):
    n_outer = slots_total // S
    assert n_outer * S == slots_total

    nc = bacc.Bacc(None, target_bir_lowering=False, debug=False)
    x_in = nc.dram_tensor("x", [P, slots_total, 3], DT.float32, kind="ExternalInput")
    tabx = nc.dram_tensor("tabx", [TOT_SLOTS, 64], DT.int32, kind="ExternalInput")
    cvec_in = nc.dram_tensor("cvec", [1, 64], DT.float32, kind="ExternalInput")
    out = nc.dram_tensor("out", [P, slots_total, 32], DT.float32, kind="ExternalOutput")

    GD_FLAT = 132 * S  # two dense halves of 2S*33, also covers 2 hash halves 2*64S

    with tile.TileContext(nc) as tc:
        with (
            tc.tile_pool(name="big", bufs=1) as bigp,
            tc.tile_pool(name="work", bufs=1) as workp,
        ):
            cv = bigp.tile([P, 64], DT.float32, tag="cv", name="cv")
            nc.sync.dma_start(cv[:], apd(cvec_in[:], 0, [[0, P], [1, 64]]))

            def cvb3(col, n, w=8):
                return apd(cv, col, [list(cv.ap[0]), [0, n], [1, w]])

            def cvb4(col, a, b, w=8):
                return apd(cv, col, [list(cv.ap[0]), [0, a], [0, b], [1, w]])

            x_t = bigp.tile([P, S, 3], DT.float32, tag="x_t", name="x_t")
            w_bf = bigp.tile([P, S, 48], DT.bfloat16, tag="w_bf", name="w_bf")
            idxb = bigp.tile([P, NCC, S], DT.int16, tag="idxb", name="idxb")
            call = bigp.tile([P, NCOFF, S], DT.int16, tag="call", name="call")
            scr = bigp.tile([P, NCC, S], DT.int16, tag="scr", name="scr", space="DRAM")
            wrpt = [
                bigp.tile([P, 64 * S], DT.int16, tag=f"wrp{i}", name=f"wrp{i}")
                for i in range(2)
            ]
            wtmp = [
                bigp.tile([P, 72 * S], DT.int16, tag=f"wtmp{i}", name=f"wtmp{i}")
                for i in range(2)
            ]
            gdt = [
                bigp.tile([P, GD_FLAT], DT.int32, tag=f"gd{i}", name=f"gd{i}")
                for i in range(2)
            ]
            osb = bigp.tile([P, S, 32], DT.float32, tag="osb", name="osb")
            bli_d = bigp.tile([P, S, 3, 8], DT.int32, tag="bli_d", name="bli_d")
            bli_h = bigp.tile([P, S, 3, 8], DT.int32, tag="bli_h", name="bli_h")

            nc.vector.memset(wrpt[0][:], 0)
            nc.vector.memset(wrpt[1][:], 0)

            with tc.For_i(
                0,
                slots_total,
                S,
                hint_engines=(mybir.EngineType.DVE, mybir.EngineType.Pool),
            ) as st:
                nc.gpsimd.dma_start(x_t[:], x_in[:, bass.ds(st, S), :])

                # ================= phase 1: indices + weights =================
                xc = workp.tile([P, S, 3], DT.float32, tag="xc", name="xc")
                nc.vector.tensor_scalar(out=xc[:], in0=x_t[:], op0=AL.max, scalar1=-1.0, op1=AL.min, scalar2=1.0)

                for half, lv0 in ((0, 0), (1, 8)):
                    bli = bli_d if half == 0 else bli_h
                    tf = workp.tile([P, S, 3, 8], DT.float32, tag="tf", name="tf")
                    fi = workp.tile([P, S, 3, 8], DT.int32, tag="fi", name="fi")
                    ff = workp.tile([P, S, 3, 8], DT.float32, tag="ff", name="ff")
                    blf = workp.tile([P, S, 3, 8], DT.float32, tag="blf", name="blf")
                    su = workp.tile([P, S, 3, 8], DT.float32, tag="su", name="su")
                    xb = apd(xc, 0, [list(xc.ap[0]), [3, S], [1, 3], [0, 8]])
                    xbu = apd(x_t, 0, [list(x_t.ap[0]), [3, S], [1, 3], [0, 8]])
                    nc.vector.tensor_scalar(out=tf[:], in0=xb, op0=AL.add, scalar1=1.0, scalar2=None)
                    nc.vector.tensor_tensor(out=tf[:], in0=tf[:], in1=cvb4(lv0, S, 3), op=AL.mult)
                    nc.vector.tensor_copy(out=fi[:], in_=tf[:])
                    nc.vector.tensor_copy(out=ff[:], in_=fi[:])
                    nc.vector.tensor_tensor(out=blf[:], in0=ff[:], in1=tf[:], op=AL.is_gt)
                    nc.vector.tensor_tensor(out=blf[:], in0=ff[:], in1=blf[:], op=AL.subtract)
                    nc.vector.tensor_scalar(out=blf[:], in0=blf[:], op0=AL.max, scalar1=0.0, scalar2=None)
                    nc.vector.tensor_tensor(out=blf[:], in0=blf[:], in1=cvb4(16 + lv0, S, 3), op=AL.min)
                    nc.vector.tensor_copy(out=bli[:], in_=blf[:])
                    nc.vector.tensor_tensor(out=su[:], in0=blf[:], in1=cvb4(32 + lv0, S, 3), op=AL.mult)
                    nc.vector.tensor_scalar(out=su[:], in0=su[:], op0=AL.add, scalar1=-1.0, scalar2=None)
                    nc.vector.tensor_tensor(out=su[:], in0=xbu, in1=su[:], op=AL.subtract)
                    nc.vector.tensor_tensor(out=su[:], in0=su[:], in1=cvb4(lv0, S, 3), op=AL.mult)
                    wdst = apd(w_bf, 3 * lv0, [list(w_bf.ap[0]), [48, S], [1, 3], [3, 8]])
                    nc.vector.tensor_copy(out=wdst, in_=su[:])

                # ---- dense pair positions ----
                i_ = bli_d[:, :, 0, :]
                j_ = bli_d[:, :, 1, :]
                k_ = bli_d[:, :, 2, :]
                p00 = workp.tile([P, S, 8], DT.int32, tag="p00", name="p00")
                p01 = workp.tile([P, S, 8], DT.int32, tag="p01", name="p01")
                p10 = workp.tile([P, S, 8], DT.int32, tag="p10", name="p10")
                p11 = workp.tile([P, S, 8], DT.int32, tag="p11", name="p11")
                nc.vector.tensor_tensor(out=p00[:], in0=j_, in1=cvb3(48, S), op=AL.mult)
                nc.vector.tensor_tensor(out=p00[:], in0=p00[:], in1=k_, op=AL.add)
                nc.vector.tensor_tensor(out=p10[:], in0=i_, in1=cvb3(56, S), op=AL.mult)
                nc.vector.tensor_tensor(out=p00[:], in0=p00[:], in1=p10[:], op=AL.add)
                nc.vector.tensor_tensor(out=p01[:], in0=p00[:], in1=cvb3(48, S), op=AL.add)
                nc.vector.tensor_tensor(out=p10[:], in0=p00[:], in1=cvb3(56, S), op=AL.add)
                nc.vector.tensor_tensor(out=p11[:], in0=p10[:], in1=cvb3(48, S), op=AL.add)
                pcls = [p00, p01, p10, p11]

                t32 = workp.tile([P, S], DT.int32, tag="t32", name="t32")
                for l in DENSE_LV:
                    G = LEVEL_G[l]
                    g = G.bit_length() - 1
                    for m in range(4):
                        src = pcls[m][:, :, l]
                        bdst = idxb[:, CC0[l] + m, :]
                        if G == 1:
                            nc.vector.tensor_copy(out=bdst, in_=src)
                        else:
                            nc.vector.tensor_scalar(out=t32[:], in0=src, op0=AL.logical_shift_right, scalar1=g, scalar2=None)
                            nc.vector.tensor_copy(out=bdst, in_=t32[:])
                            nc.vector.tensor_scalar(out=t32[:], in0=src, op0=AL.bitwise_and, scalar1=G - 1, scalar2=None)
                            nc.vector.tensor_copy(out=call[:, CCOFF[l] + m, :], in_=t32[:])

                # ---- hash rows (vectorized over the 8 hash levels) ----
                ih = bli_h[:, :, 0, :]
                jh = bli_h[:, :, 1, :]
                kh = bli_h[:, :, 2, :]
                mt1 = workp.tile([P, S, 8], DT.int32, tag="mt1", name="mt1")
                mt2 = workp.tile([P, S, 8], DT.int32, tag="mt2", name="mt2")
                mt3 = workp.tile([P, S, 8], DT.int32, tag="mt3", name="mt3")

                def ts(o, i, op, s):
                    nc.vector.tensor_scalar(out=o, in0=i, op0=op, scalar1=s, scalar2=None)

                def tt(o, a, b, op):
                    nc.vector.tensor_tensor(out=o, in0=a, in1=b, op=op)

                def mul32(dst, src, prime):
                    Hp, Lp = (prime >> 16) & 0xFFFF, prime & 0xFFFF
                    Hs = Hp - 32768 if Hp >= 32768 else Hp
                    ts(mt1[:], src, AL.mult, Lp)
                    ts(mt2[:], src, AL.mult, Hs)
                    if Hp >= 32768:
                        ts(mt3[:], src, AL.logical_shift_left, 15)
                        ts(mt3[:], mt3[:], AL.bitwise_and, 0xFFFF)
                        ts(mt2[:], mt2[:], AL.bitwise_and, 0xFFFF)
                        tt(mt2[:], mt2[:], mt3[:], AL.add)
                    ts(mt2[:], mt2[:], AL.bitwise_and, 0xFFFF)
                    ts(mt3[:], mt1[:], AL.logical_shift_right, 16)
                    tt(mt2[:], mt2[:], mt3[:], AL.add)
                    ts(mt2[:], mt2[:], AL.bitwise_and, 0xFFFF)
                    ts(mt2[:], mt2[:], AL.logical_shift_left, 16)
                    ts(mt1[:], mt1[:], AL.bitwise_and, 0xFFFF)
                    tt(dst, mt2[:], mt1[:], AL.bitwise_or)

                def add32(dst, src, const):
                    cl, ch = const & 0xFFFF, (const >> 16) & 0xFFFF
                    ts(mt1[:], src, AL.bitwise_and, 0xFFFF)
                    ts(mt1[:], mt1[:], AL.add, cl)
                    ts(mt2[:], src, AL.logical_shift_right, 16)
                    ts(mt2[:], mt2[:], AL.bitwise_and, 0xFFFF)
                    ts(mt2[:], mt2[:], AL.add, ch)
                    ts(mt3[:], mt1[:], AL.logical_shift_right, 16)
                    tt(mt2[:], mt2[:], mt3[:], AL.add)
                    ts(mt2[:], mt2[:], AL.bitwise_and, 0xFFFF)
                    ts(mt2[:], mt2[:], AL.logical_shift_left, 16)
                    ts(mt1[:], mt1[:], AL.bitwise_and, 0xFFFF)
                    tt(dst, mt2[:], mt1[:], AL.bitwise_or)

                jp0 = workp.tile([P, S, 8], DT.int32, tag="jp0", name="jp0")
                jp1 = workp.tile([P, S, 8], DT.int32, tag="jp1", name="jp1")
                kp0 = workp.tile([P, S, 8], DT.int32, tag="kp0", name="kp0")
                kp1 = workp.tile([P, S, 8], DT.int32, tag="kp1", name="kp1")
                ii1 = workp.tile([P, S, 8], DT.int32, tag="ii1", name="ii1")
                rr = workp.tile([P, S, 8], DT.int32, tag="rr", name="rr")
                rr2 = workp.tile([P, S, 8], DT.int32, tag="rr2", name="rr2")
                mul32(jp0[:], jh, PRIMES[1])
                add32(jp1[:], jp0[:], PRIMES[1])
                mul32(kp0[:], kh, PRIMES[2])
                add32(kp1[:], kp0[:], PRIMES[2])
                ts(ii1[:], ih, AL.add, 1)
                for m in range(8):
                    di, dj, dk = (m >> 2) & 1, (m >> 1) & 1, m & 1
                    tt(rr[:], ii1[:] if di else ih, (jp1 if dj else jp0)[:], AL.bitwise_xor)
                    tt(rr[:], rr[:], (kp1 if dk else kp0)[:], AL.bitwise_xor)
                    ts(rr[:], rr[:], AL.bitwise_and, T - 1)
                    rsrc = apd(rr2, 0, [list(rr2.ap[0]), [1, 8], [8, S]])
                    ts(rr2[:], rr[:], AL.logical_shift_right, 4)
                    bap = apd(idxb, (CC0[8] + m) * S, [list(idxb.ap[0]), [8 * S, 8], [1, S]])
                    nc.vector.tensor_copy(out=bap, in_=rsrc)
                    ts(rr2[:], rr[:], AL.bitwise_and, 15)
                    cap = apd(call, (CCOFF[8] + m) * S, [list(call.ap[0]), [8 * S, 8], [1, S]])
                    nc.vector.tensor_copy(out=cap, in_=rsrc)

                # ================= phase 2: roundtrip, gathers, interp =========
                nc.sync.dma_start(scr[:], idxb[:])

                def wrapped_read(wrp, wt, cc0, ncls):
                    # DRAM->SBUF in t-major blocks (contiguous runs of ncls*S
                    # int16 -> sane descriptors), then lane-interleave on Pool:
                    # wrp[q, ccs*8+t] = wt[q, t*nS+ccs].
                    nS = ncls * S
                    src_dims = [[NCC * S, 16], [16 * NCC * S, 8], [1, nS]]
                    for q0 in (0, 16):
                        nc.sync.dma_start(
                            wt[q0 : q0 + 16, 0 : 8 * nS], apd(scr, cc0 * S, src_dims)
                        )
                    obase = wrp[0:32, 0 : 8 * nS]
                    oap = bass.AP(
                        obase.tensor, obase.offset, [list(obase.ap[0]), [8, nS], [1, 8]]
                    )
                    ibase = wt[0:32, 0 : 8 * nS]
                    iap = bass.AP(
                        ibase.tensor, ibase.offset, [list(ibase.ap[0]), [1, nS], [nS, 8]]
                    )
                    nc.gpsimd.tensor_copy(out=oap, in_=iap)

                def lerp(dst, lo, hi, w):
                    nc.vector.tensor_tensor(out=dst, in0=hi, in1=lo, op=AL.subtract)
                    nc.vector.tensor_tensor(out=dst, in0=dst, in1=w, op=AL.mult)
                    nc.vector.tensor_tensor(out=dst, in0=dst, in1=lo, op=AL.add)

                def unpack(src_i32_ap, n, tag):
                    f0 = workp.tile([P, n], DT.int32, tag=f"u0{tag}", name=f"u0{tag}")
                    f1 = workp.tile([P, n], DT.int32, tag=f"u1{tag}", name=f"u1{tag}")
                    nc.vector.tensor_scalar(out=f0[:], in0=src_i32_ap, op0=AL.logical_shift_left, scalar1=16, scalar2=None)
                    nc.vector.tensor_scalar(out=f1[:], in0=src_i32_ap, op0=AL.bitwise_and, scalar1=_i32(0xFFFF0000), scalar2=None)
                    return f0, f1

                def f32v(t, a, n):
                    return apd(t, a, [list(t.ap[0]), [1, n]]).bitcast(DT.float32)

                def mk_masks(cc_off, ncls, nbits):
                    cflat = workp.tile([P, ncls * S], DT.int32, tag="cf", name="cf")
                    nc.vector.tensor_copy(
                        out=cflat[:], in_=apd(call, cc_off * S, [list(call.ap[0]), [1, ncls * S]])
                    )
                    ms = []
                    for b in range(nbits):
                        mb = workp.tile([P, ncls * S], DT.int32, tag=f"mk{b}", name=f"mk{b}")
                        nc.vector.tensor_scalar(out=mb[:], in0=cflat[:], op0=AL.logical_shift_right, scalar1=b, op1=AL.bitwise_and, scalar2=1)
                        nc.vector.tensor_scalar(out=mb[:], in0=mb[:], op0=AL.mult, scalar1=-1, scalar2=None)
                        ms.append(mb)
                    return ms

                def seltree(gd, base_off, el, lane_off, n, masks, nbits, tag):
                    cur = None
                    for b in range(nbits - 1, -1, -1):
                        half = 1 << b
                        nxt = workp.tile([P, n, half], DT.int32, tag=f"st{tag}{b}", name=f"st{tag}{b}")
                        if cur is None:
                            lo = apd(gd, base_off + lane_off, [list(gd.ap[0]), [el, n], [1, half]])
                            hi = apd(gd, base_off + lane_off + half, [list(gd.ap[0]), [el, n], [1, half]])
                        else:
                            lo = cur[:, :, 0:half]
                            hi = cur[:, :, half : 2 * half]
                        m = masks[b]
                        mb = apd(m, 0, [list(m.ap[0]), [1, n], [0, half]])
                        nc.vector.tensor_tensor(out=nxt[:], in0=lo, in1=hi, op=AL.bitwise_xor)
                        nc.vector.tensor_tensor(out=nxt[:], in0=nxt[:], in1=mb, op=AL.bitwise_and)
                        nc.vector.tensor_tensor(out=nxt[:], in0=nxt[:], in1=lo, op=AL.bitwise_xor)
                        cur = nxt
                    return cur

                def wof(l, d, reps):
                    wf = workp.tile([P, S], DT.float32, tag=f"wf{d}", name=f"wf{d}")
                    nc.vector.tensor_copy(out=wf[:], in_=apd(w_bf, 3 * l + d, [list(w_bf.ap[0]), [48, S]]))
                    return apd(wf, 0, [list(wf.ap[0]), [0, reps], [1, S]])

                for l in range(N_LEVELS):
                    G = LEVEL_G[l]
                    g = G.bit_length() - 1
                    el = ELEM[l]
                    gd = gdt[l % 2]
                    wrp = wrpt[l % 2]
                    base = tabx[SLOT_OFF[l] : SLOT_OFF[l] + N_SLOTS[l], 0:el]
                    if l in HASH_LV:
                        wrapped_read(wrp, wtmp[l % 2], CC0[l], 8)
                        n4 = 4 * S
                        for h in range(2):
                            dma_gather_chunked(nc.gpsimd, gd, h * n4 * el, el, base, wrp, h * 32 * S, n4 * P)
                        ex = []
                        for h in range(2):
                            masks = mk_masks(CCOFF[l] + 4 * h, 4, 4)
                            ex.append(seltree(gd, h * n4 * el, el, 0, n4, masks, 4, f"h{h}"))
                        e0f0, e0f1 = unpack(ex[0][:, :, 0], n4, "a")
                        e1f0, e1f1 = unpack(ex[1][:, :, 0], n4, "b")
                        wx = wof(l, 0, 4)
                        x0 = workp.tile([P, 4 * S], DT.float32, tag="x0", name="x0")
                        x1 = workp.tile([P, 4 * S], DT.float32, tag="x1", name="x1")
                        lerp(x0[:], f32v(e0f0, 0, n4), f32v(e1f0, 0, n4), wx)
                        lerp(x1[:], f32v(e0f1, 0, n4), f32v(e1f1, 0, n4), wx)
                        wy = wof(l, 1, 2)
                        y0 = workp.tile([P, 2 * S], DT.float32, tag="y0", name="y0")
                        y1 = workp.tile([P, 2 * S], DT.float32, tag="y1", name="y1")
                        lerp(y0[:], x0[:, 0 : 2 * S], x0[:, 2 * S : 4 * S], wy)
                        lerp(y1[:], x1[:, 0 : 2 * S], x1[:, 2 * S : 4 * S], wy)
                        wz = wof(l, 2, 1)
                        o0 = osb[:, :, 2 * l : 2 * l + 1].rearrange("p s o -> p (s o)")
                        o1 = osb[:, :, 2 * l + 1 : 2 * l + 2].rearrange("p s o -> p (s o)")
                        lerp(o0, y0[:, 0:S], y0[:, S : 2 * S], wz)
                        lerp(o1, y1[:, 0:S], y1[:, S : 2 * S], wz)
                    else:
                        wrapped_read(wrp, wtmp[l % 2], CC0[l], 4)
                        if G == 1:
                            n4 = 4 * S
                            dma_gather_chunked(nc.gpsimd, gd, 0, el, base, wrp, 0, n4 * P)
                            A = apd(gd, 0, [list(gd.ap[0]), [2, n4]])
                            B = apd(gd, 1, [list(gd.ap[0]), [2, n4]])
                            a0 = unpack(A, n4, "a")  # dk=0: (f0 tile, f1 tile) [P, 4S]
                            a1 = unpack(B, n4, "b")  # dk=1
                            # di split: [0:2S] di0, [2S:4S] di1
                            pairs = [
                                (f32v(a0[0], 0, 2 * S), f32v(a0[0], 2 * S, 2 * S)),
                                (f32v(a0[1], 0, 2 * S), f32v(a0[1], 2 * S, 2 * S)),
                                (f32v(a1[0], 0, 2 * S), f32v(a1[0], 2 * S, 2 * S)),
                                (f32v(a1[1], 0, 2 * S), f32v(a1[1], 2 * S, 2 * S)),
                            ]
                        else:
                            n2 = 2 * S
                            for h in range(2):
                                dma_gather_chunked(nc.gpsimd, gd, h * n2 * el, el, base, wrp, h * 16 * S, n2 * P)
                            exh = []
                            for h in range(2):
                                masks = mk_masks(CCOFF[l] + 2 * h, 2, g)
                                eA = seltree(gd, h * n2 * el, el, 0, n2, masks, g, f"A{h}")
                                eB = seltree(gd, h * n2 * el, el, 1, n2, masks, g, f"B{h}")
                                exh.append((eA, eB))
                            a00 = unpack(exh[0][0][:, :, 0], n2, "a")  # h0 dk0
                            a01 = unpack(exh[1][0][:, :, 0], n2, "b")  # h1 dk0
                            a10 = unpack(exh[0][1][:, :, 0], n2, "c")  # h0 dk1
                            a11 = unpack(exh[1][1][:, :, 0], n2, "d")  # h1 dk1
                            pairs = [
                                (f32v(a00[0], 0, 2 * S), f32v(a01[0], 0, 2 * S)),
                                (f32v(a00[1], 0, 2 * S), f32v(a01[1], 0, 2 * S)),
                                (f32v(a10[0], 0, 2 * S), f32v(a11[0], 0, 2 * S)),
                                (f32v(a10[1], 0, 2 * S), f32v(a11[1], 0, 2 * S)),
                            ]
                        # pairs rows: (dk0_f0, dk0_f1, dk1_f0, dk1_f1) with (lo=di0, hi=di1)
                        wx = wof(l, 0, 2)
                        xt = [
                            workp.tile([P, 2 * S], DT.float32, tag=f"x{q}", name=f"x{q}")
                            for q in range(4)
                        ]
                        for q in range(4):
                            lerp(xt[q][:], pairs[q][0], pairs[q][1], wx)
                        wy = wof(l, 1, 1)
                        yt = [
                            workp.tile([P, S], DT.float32, tag=f"y{q}", name=f"y{q}")
                            for q in range(4)
                        ]
                        for q in range(4):
                            lerp(yt[q][:], xt[q][:, 0:S], xt[q][:, S : 2 * S], wy)
                        wz = wof(l, 2, 1)
                        o0 = osb[:, :, 2 * l : 2 * l + 1].rearrange("p s o -> p (s o)")
                        o1 = osb[:, :, 2 * l + 1 : 2 * l + 2].rearrange("p s o -> p (s o)")
                        lerp(o0, yt[0][:], yt[2][:], wz)
                        lerp(o1, yt[1][:], yt[3][:], wz)

                nc.gpsimd.dma_start(out[:, bass.ds(st, S), :], osb[:])
    nc.compile()
    return nc


# ---------------- host side ----------------

def _pack_bf16(t):
    u = np.ascontiguousarray(t.astype(np.float32)).view(np.uint32).astype(np.uint64)
    r = ((u + 0x7FFF + ((u >> 16) & 1)) >> 16).astype(np.uint32)
    return r[:, 0] | (r[:, 1] << np.uint32(16))


def build_tabx(tables):
    pk = _pack_bf16(tables)
    tabx = np.zeros((TOT_SLOTS, 64), dtype=np.uint32)
    for l in range(N_LEVELS):
        G = LEVEL_G[l]
        o, ns = SLOT_OFF[l], N_SLOTS[l]
        seg = pk[OFFS[l] : OFFS[l + 1]]
        if G == 1:
            tabx[o : o + ns, 0] = seg
            tabx[o : o + ns - 1, 1] = seg[1:]
            tabx[o + ns - 1, 1] = seg[-1]
        elif l in DENSE_LV:
            buf = np.zeros(ns * G + G, dtype=np.uint32)
            buf[: len(seg)] = seg
            for j in range(G + 1):
                tabx[o : o + ns, j] = buf[j : j + ns * G : G]
        else:
            tabx[o : o + ns, 0:16] = seg.reshape(ns, 16)
    return tabx.view(np.int32)


def build_cvec():
    cv = np.zeros(64, dtype=np.float32)
    for l in range(N_LEVELS):
        grid = np.float32(2.0) / np.float32(RES[l])
        cv[l] = np.float32(1.0) / grid
        cv[16 + l] = np.float32(RES[l] - 1)
        cv[32 + l] = grid
    for l in range(8):
        cv[48 + l] = np.float32(RES[l])
        cv[56 + l] = np.float32(RES[l] * RES[l])
    return cv.reshape(1, 64)


_NC_CACHE = {}
TRACE = False
LAST_NS = None


def _get_nc(slots, S):
    key = (slots, S)
    if key not in _NC_CACHE:
        _NC_CACHE[key] = build_kernel(slots, S)
    return _NC_CACHE[key]


def kernel(x: np.ndarray, tables: np.ndarray) -> np.ndarray:
    global LAST_NS
    from concourse.bass_utils import run_bass_kernel_spmd

    B = x.shape[0]
    per_core = B // N_CORES
    slots = per_core // P
    S = min(32, slots)
    nc = _get_nc(slots, S)
    tabx = build_tabx(tables)
    cv = build_cvec()
    in_maps = []
    for c in range(N_CORES):
        xs = np.ascontiguousarray(
            x[c * per_core : (c + 1) * per_core].reshape(P, slots, 3)
        ).astype(np.float32)
        in_maps.append({"x": xs, "tabx": tabx, "cvec": cv})
    kw = {"trace": True} if TRACE else {}
    res = run_bass_kernel_spmd(nc, in_maps, core_ids=list(range(N_CORES)), **kw)
    LAST_NS = res.exec_time_ns
    outs = [res.results[c]["out"].reshape(per_core, 32) for c in range(N_CORES)]
    return np.concatenate(outs, axis=0).astype(np.float32)



# revision 7
# speedup vs baseline: 1.8470x; 1.8470x over previous
"""HashEmbedder3D Trainium2 kernel v6.

Key changes vs v2 baseline:
- dma_gather with single_packet=False + indices replicated across all 8
  16-partition groups allows 8192-idx instructions (994ns fixed cost
  amortized 8x).
- Dense levels 0-3: slot-per-voxel-base block tables (one 32B descriptor
  fetches all 8 corners; no select trees). Levels 4-7: Q-packed slots
  (one descriptor + small k-offset select tree).
- Hash levels 8-15: one 64B slot read per corner (8 classes), extraction
  via 4-round select tree, gathers batched 2 classes per instruction.
- Index wrap/transpose done by DRAM-bounce DMA + DVE interleave + SBUF
  broadcast DMAs; Pool engine only runs SWDGE gathers.
"""
import math
import sys

import numpy as np

sys.path.insert(0, "/opt/trn_rl_repo")

from concourse import bacc, bass, mybir
import concourse.tile as tile

N_LEVELS = 16
F = 2
LOG2_T = 19
T = 1 << LOG2_T
BASE, FINEST = 16, 512
B_GROWTH = float(np.exp((np.log(np.float32(FINEST)) - np.log(np.float32(BASE))) / np.float32(N_LEVELS - 1)))
RES = [math.floor(BASE * B_GROWTH**i) for i in range(N_LEVELS)]
SIZES = [(r + 1) ** 3 if r**3 < T else T for r in RES]
OFFS = np.concatenate([[0], np.cumsum(SIZES)]).tolist()
TOTAL_ROWS = OFFS[-1]
PRIMES = [1, 2654435761, 805459861]
N_POINTS = 1048576
N_CORES = 8
P = 128

DT = mybir.dt
AL = mybir.AluOpType

DENSE_LV = list(range(8))
HASH_LV = list(range(8, 16))

# dense level slot geometry: levels 0-3 slot-per-base, 4-7 Q-packed
LEVEL_Q = {0: 1, 1: 1, 2: 1, 3: 1, 4: 2, 5: 4, 6: 8, 7: 16}
LEVEL_M = {}
N_SLOTS, ELEM = {}, {}
for l in DENSE_LV:
    r, Q = RES[l], LEVEL_Q[l]
    M = -(-r // Q)
    LEVEL_M[l] = M
    N_SLOTS[l] = r * r * M
    ELEM[l] = 8 if Q == 1 else 4 * (Q + 1)
ELEM[7] = 2 * (LEVEL_Q[7] + 1)  # level 7 split in two di-halves of 2x17 lanes
for l in HASH_LV:
    N_SLOTS[l], ELEM[l] = T // 16, 16
SLOT_OFF = {}
_a = 0
for l in range(N_LEVELS):
    SLOT_OFF[l] = _a
    _a += N_SLOTS[l]
SLOT_OFF7B = _a  # second (di=1) half of level 7
_a += N_SLOTS[7]
TOT_SLOTS = _a
assert all(N_SLOTS[l] <= 32768 for l in range(N_LEVELS)), N_SLOTS

# class layout: dense levels are classes 0-7; hash level l corner m is
# class 8 + (l-8)*8 + m. Each class contributes S indices per tile.
NCC = 8 + 8 * 8  # 72


def _i32(v):
    return int(np.int32(np.uint32(v)))


MAX_GIDX = 8192


def dma_gather_raw(eng, out_ap, in_ap, idxs_ap, num_idxs, elem_size, elem_step=64):
    stride_bytes = elem_step * 4
    assert stride_bytes % 256 == 0
    _in_ap = eng.lower_ap_dma(in_ap, for_custom_bir_dma=True)
    _idxs_ap = eng.lower_ap(idxs_ap)
    _out_ap = eng.lower_ap(out_ap)
    return eng.add_instruction(
        mybir.InstDMAGatherAnt(
            name=eng.bass.get_next_instruction_name(),
            ins=[*_in_ap, _idxs_ap, eng.lower_val_access(eng.to_reg(num_idxs))],
            outs=[_out_ap],
            transpose=False,
            num_idxs=num_idxs,
            elem_size=elem_size,
            stride_bytes_256=stride_bytes // 256,
            gen_mode=0,
            single_packet=num_idxs <= 1024,
            queue_num=0,
            sbuf_tokens_per_rank=0,
            sbuf_free_dim_per_rank=0,
            sbuf_free_dim_pad_per_rank=0,
            sbuf_byte_offset=0,
        )
    )


def apd(tap, off, dims):
    return bass.AP(tap.tensor, tap.offset + off, [list(d) for d in dims])


def build_kernel(slots_total, S):
    n_outer = slots_total // S
    assert n_outer * S == slots_total

    nc = bacc.Bacc(None, target_bir_lowering=False, debug=False)
    x_in = nc.dram_tensor("x", [P, slots_total, 3], DT.float32, kind="ExternalInput")
    tabx = nc.dram_tensor("tabx", [TOT_SLOTS, 64], DT.int32, kind="ExternalInput")
    cvec_in = nc.dram_tensor("cvec", [1, 128], DT.float32, kind="ExternalInput")
    out = nc.dram_tensor("out", [P, slots_total, 32], DT.float32, kind="ExternalOutput")

    nW = NCC * S  # idx ints per partition per tile

    with tile.TileContext(nc) as tc:
        with (
            tc.tile_pool(name="big", bufs=1) as bigp,
            tc.tile_pool(name="dbl", bufs=1) as dblp,
            tc.tile_pool(name="work", bufs=1) as workp,
        ):
            cv = bigp.tile([P, 128], DT.float32, tag="cv", name="cv")
            nc.sync.dma_start(cv[:], apd(cvec_in[:], 0, [[0, P], [1, 128]]))

            def cvb3(col, n, w=8):
                # [P, n, w] view of per-level const at cv[col:col+w]
                return apd(cv, col, [list(cv.ap[0]), [0, n], [1, w]])

            def cvb4(col, a, b, w=8):
                return apd(cv, col, [list(cv.ap[0]), [0, a], [0, b], [1, w]])

            x_t = bigp.tile([P, S, 3], DT.float32, tag="x_t", name="x_t")
            w_bf = None
            idxb = bigp.tile([P, nW], DT.int16, tag="idxb", name="idxb")
            hm = None
            koffs = None
            scr = bigp.tile([P, nW], DT.int16, tag="scr", name="scr", space="DRAM")
            wt = bigp.tile([P, 2 * nW], DT.int16, tag="wt", name="wt")
            wrp = None
            gdd = [
                bigp.tile([P, 68 * S], DT.int32, tag=f"gdd{i}", name=f"gdd{i}")
                for i in range(2)
            ]
            gdh = [
                bigp.tile([P, 2 * 16 * S], DT.int32, tag=f"gdh{i}", name=f"gdh{i}")
                for i in range(3)
            ]
            hcor = bigp.tile([P, 8 * S], DT.int32, tag="hcor", name="hcor")
            osb = bigp.tile([P, S, 32], DT.float32, tag="osb", name="osb")
            bli_d = bigp.tile([P, S, 3, 8], DT.int32, tag="bli_d", name="bli_d")
            bli_h = bigp.tile([P, S, 3, 8], DT.int32, tag="bli_h", name="bli_h")

            nc.vector.memset(wt[:], 0)

            with tc.For_i(
                0,
                slots_total,
                S,
                hint_engines=(mybir.EngineType.DVE, mybir.EngineType.Pool),
            ) as st:
                nc.sync.dma_start(x_t[:], x_in[:, bass.ds(st, S), :])
                w_bf = dblp.tile([P, S, 48], DT.bfloat16, tag="w_bf", name="w_bf")
                hm = dblp.tile([P, 64 * S], DT.int32, tag="hm", name="hm")
                koffs = dblp.tile([P, S, 8], DT.int32, tag="koffs", name="koffs")
                wrp = dblp.tile([P, 8 * nW], DT.int16, tag="wrp", name="wrp")

                # ================= phase 1: voxel coords + weights ============
                xc = workp.tile([P, S, 3], DT.float32, tag="xc", name="xc")
                nc.vector.tensor_scalar(out=xc[:], in0=x_t[:], op0=AL.max, scalar1=-1.0, op1=AL.min, scalar2=1.0)

                for half, lv0 in ((0, 0), (1, 8)):
                    bli = bli_d if half == 0 else bli_h
                    tf = workp.tile([P, S, 3, 8], DT.float32, tag="tf", name="tf")
                    fi = workp.tile([P, S, 3, 8], DT.int32, tag="fi", name="fi")
                    ff = workp.tile([P, S, 3, 8], DT.float32, tag="ff", name="ff")
                    blf = workp.tile([P, S, 3, 8], DT.float32, tag="blf", name="blf")
                    su = workp.tile([P, S, 3, 8], DT.float32, tag="su", name="su")
                    xb = apd(xc, 0, [list(xc.ap[0]), [3, S], [1, 3], [0, 8]])
                    xbu = apd(x_t, 0, [list(x_t.ap[0]), [3, S], [1, 3], [0, 8]])
                    nc.vector.tensor_scalar(out=tf[:], in0=xb, op0=AL.add, scalar1=1.0, scalar2=None)
                    nc.vector.tensor_tensor(out=tf[:], in0=tf[:], in1=cvb4(lv0, S, 3), op=AL.mult)
                    nc.vector.tensor_copy(out=fi[:], in_=tf[:])
                    nc.vector.tensor_copy(out=ff[:], in_=fi[:])
                    nc.vector.tensor_tensor(out=blf[:], in0=ff[:], in1=tf[:], op=AL.is_gt)
                    nc.vector.tensor_tensor(out=blf[:], in0=ff[:], in1=blf[:], op=AL.subtract)
                    nc.vector.tensor_scalar(out=blf[:], in0=blf[:], op0=AL.max, scalar1=0.0, scalar2=None)
                    nc.vector.tensor_tensor(out=blf[:], in0=blf[:], in1=cvb4(16 + lv0, S, 3), op=AL.min)
                    nc.vector.tensor_copy(out=bli[:], in_=blf[:])
                    nc.vector.tensor_tensor(out=su[:], in0=blf[:], in1=cvb4(32 + lv0, S, 3), op=AL.mult)
                    nc.vector.tensor_scalar(out=su[:], in0=su[:], op0=AL.add, scalar1=-1.0, scalar2=None)
                    nc.vector.tensor_tensor(out=su[:], in0=xbu, in1=su[:], op=AL.subtract)
                    nc.vector.tensor_tensor(out=su[:], in0=su[:], in1=cvb4(lv0, S, 3), op=AL.mult)
                    wdst = apd(w_bf, 3 * lv0, [list(w_bf.ap[0]), [48, S], [1, 3], [3, 8]])
                    nc.vector.tensor_copy(out=wdst, in_=su[:])

                # ================= dense slot ids =============================
                i_ = bli_d[:, :, 0, :]
                j_ = bli_d[:, :, 1, :]
                k_ = bli_d[:, :, 2, :]
                kq = workp.tile([P, S, 8], DT.int32, tag="kq", name="kq")
                sid = workp.tile([P, S, 8], DT.int32, tag="sid", name="sid")
                t1 = workp.tile([P, S, 8], DT.int32, tag="t1d", name="t1d")
                nc.vector.tensor_copy(out=kq[:], in_=k_)
                for l in range(4, 8):
                    q = LEVEL_Q[l].bit_length() - 1
                    nc.vector.tensor_scalar(out=kq[:, :, l], in0=k_[:, :, l], op0=AL.logical_shift_right, scalar1=q, scalar2=None)
                # koff = k - kq*Q  (only levels 4-7 used)
                nc.vector.tensor_tensor(out=koffs[:], in0=kq[:], in1=apd(cv, 72, [list(cv.ap[0]), [0, S], [1, 8]]), op=AL.mult)
                nc.vector.tensor_tensor(out=koffs[:], in0=k_, in1=koffs[:], op=AL.subtract)
                # sid = i*A + j*B + kq
                nc.vector.tensor_tensor(out=t1[:], in0=i_, in1=apd(cv, 48, [list(cv.ap[0]), [0, S], [1, 8]]), op=AL.mult)
                nc.vector.tensor_tensor(out=sid[:], in0=j_, in1=apd(cv, 56, [list(cv.ap[0]), [0, S], [1, 8]]), op=AL.mult)
                nc.vector.tensor_tensor(out=sid[:], in0=sid[:], in1=t1[:], op=AL.add)
                nc.vector.tensor_tensor(out=sid[:], in0=sid[:], in1=kq[:], op=AL.add)
                # write dense classes: idxb[:, l*S + x] = sid[:, x, l]
                nc.vector.tensor_copy(
                    out=apd(idxb, 0, [list(idxb.ap[0]), [1, S], [S, 8]]),
                    in_=sid[:],
                )

                # ================= hash slot ids ==============================
                ih = bli_h[:, :, 0, :]
                jh = bli_h[:, :, 1, :]
                kh = bli_h[:, :, 2, :]
                mt1 = workp.tile([P, S, 8], DT.int32, tag="mt1", name="mt1")
                mt2 = workp.tile([P, S, 8], DT.int32, tag="mt2", name="mt2")
                mt3 = workp.tile([P, S, 8], DT.int32, tag="mt3", name="mt3")

                def ts(o, i, op, s):
                    nc.vector.tensor_scalar(out=o, in0=i, op0=op, scalar1=s, scalar2=None)

                def tt(o, a, b, op):
                    nc.vector.tensor_tensor(out=o, in0=a, in1=b, op=op)

                def mul32(dst, src, prime):
                    Hp, Lp = (prime >> 16) & 0xFFFF, prime & 0xFFFF
                    Hs = Hp - 32768 if Hp >= 32768 else Hp
                    ts(mt1[:], src, AL.mult, Lp)
                    ts(mt2[:], src, AL.mult, Hs)
                    if Hp >= 32768:
                        ts(mt3[:], src, AL.logical_shift_left, 15)
                        ts(mt3[:], mt3[:], AL.bitwise_and, 0xFFFF)
                        ts(mt2[:], mt2[:], AL.bitwise_and, 0xFFFF)
                        tt(mt2[:], mt2[:], mt3[:], AL.add)
                    ts(mt2[:], mt2[:], AL.bitwise_and, 0xFFFF)
                    ts(mt3[:], mt1[:], AL.logical_shift_right, 16)
                    tt(mt2[:], mt2[:], mt3[:], AL.add)
                    ts(mt2[:], mt2[:], AL.bitwise_and, 0xFFFF)
                    ts(mt2[:], mt2[:], AL.logical_shift_left, 16)
                    ts(mt1[:], mt1[:], AL.bitwise_and, 0xFFFF)
                    tt(dst, mt2[:], mt1[:], AL.bitwise_or)

                def add32(dst, src, const):
                    cl, ch = const & 0xFFFF, (const >> 16) & 0xFFFF
                    ts(mt1[:], src, AL.bitwise_and, 0xFFFF)
                    ts(mt1[:], mt1[:], AL.add, cl)
                    ts(mt2[:], src, AL.logical_shift_right, 16)
                    ts(mt2[:], mt2[:], AL.bitwise_and, 0xFFFF)
                    ts(mt2[:], mt2[:], AL.add, ch)
                    ts(mt3[:], mt1[:], AL.logical_shift_right, 16)
                    tt(mt2[:], mt2[:], mt3[:], AL.add)
                    ts(mt2[:], mt2[:], AL.bitwise_and, 0xFFFF)
                    ts(mt2[:], mt2[:], AL.logical_shift_left, 16)
                    ts(mt1[:], mt1[:], AL.bitwise_and, 0xFFFF)
                    tt(dst, mt2[:], mt1[:], AL.bitwise_or)

                jp0 = workp.tile([P, S, 8], DT.int32, tag="jp0", name="jp0")
                jp1 = workp.tile([P, S, 8], DT.int32, tag="jp1", name="jp1")
                kp0 = workp.tile([P, S, 8], DT.int32, tag="kp0", name="kp0")
                kp1 = workp.tile([P, S, 8], DT.int32, tag="kp1", name="kp1")
                ii1 = workp.tile([P, S, 8], DT.int32, tag="ii1", name="ii1")
                rr = workp.tile([P, S, 8], DT.int32, tag="rr", name="rr")
                rr2 = workp.tile([P, S, 8], DT.int32, tag="rr2", name="rr2")
                mul32(jp0[:], jh, PRIMES[1])
                add32(jp1[:], jp0[:], PRIMES[1])
                mul32(kp0[:], kh, PRIMES[2])
                add32(kp1[:], kp0[:], PRIMES[2])
                ts(ii1[:], ih, AL.add, 1)
                for m in range(8):
                    di, dj, dk = (m >> 2) & 1, (m >> 1) & 1, m & 1
                    tt(rr[:], ii1[:] if di else ih, (jp1 if dj else jp0)[:], AL.bitwise_xor)
                    tt(rr[:], rr[:], (kp1 if dk else kp0)[:], AL.bitwise_xor)
                    ts(rr[:], rr[:], AL.bitwise_and, T - 1)
                    # idxb[:, (8 + (l-8)*8 + m)*S + x] = rr[:, x, l] >> 4
                    ts(rr2[:], rr[:], AL.logical_shift_right, 4)
                    nc.vector.tensor_copy(
                        out=apd(idxb, (8 + m) * S, [list(idxb.ap[0]), [1, S], [8 * S, 8]]),
                        in_=rr2[:],
                    )
                    # hm[:, (l-8)*8S + m*S + x] = rr & 15
                    ts(rr2[:], rr[:], AL.bitwise_and, 15)
                    nc.vector.tensor_copy(
                        out=apd(hm, m * S, [list(hm.ap[0]), [1, S], [8 * S, 8]]),
                        in_=rr2[:],
                    )

                # ============== idx transpose to wrapped-16 + replicate =======
                nc.sync.dma_start(scr[:], idxb[:])
                # wt[q, h*nW + col] = scr[16(h+h0)+q, col]  (q<16), four chunks
                for h0 in (0, 2, 4, 6):
                    nc.sync.dma_start(
                        wt[0:16, :],
                        apd(scr, h0 * 16 * nW, [[nW, 16], [16 * nW, 2], [1, nW]]),
                    )
                    # wrp[q, col*8 + h0+h] = wt[q, h*nW + col]
                    nc.vector.tensor_copy(
                        out=apd(wrp, h0, [list(wrp.ap[0]), [8, nW], [1, 2]]),
                        in_=apd(wt, 0, [list(wt.ap[0]), [1, nW], [nW, 2]]),
                    )
                for g in range(1, 8):
                    nc.sync.dma_start(wrp[16 * g : 16 * g + 16, :], wrp[0:16, :])

                # ================= gathers ====================================
                def lerp(dst, lo, hi, w):
                    nc.vector.tensor_tensor(out=dst, in0=hi, in1=lo, op=AL.subtract)
                    nc.vector.tensor_tensor(out=dst, in0=dst, in1=w, op=AL.mult)
                    nc.vector.tensor_tensor(out=dst, in0=dst, in1=lo, op=AL.add)

                def wof(l, d, reps, minor):
                    # weight w_bf[:, x, 3l+d] broadcast: minor=True -> [S, reps]
                    # (x outer), else [reps, S]
                    wf = workp.tile([P, S], DT.float32, tag=f"wf{d}", name=f"wf{d}")
                    nc.vector.tensor_copy(out=wf[:], in_=apd(w_bf, 3 * l + d, [list(w_bf.ap[0]), [48, S]]))
                    if minor:
                        return apd(wf, 0, [list(wf.ap[0]), [1, S], [0, reps]])
                    return apd(wf, 0, [list(wf.ap[0]), [0, reps], [1, S]])

                def unpack(src_ap, n, tag):
                    f0 = workp.tile([P, n], DT.int32, tag=f"u0{tag}", name=f"u0{tag}")
                    f1 = workp.tile([P, n], DT.int32, tag=f"u1{tag}", name=f"u1{tag}")
                    nc.vector.tensor_scalar(out=f0[:], in0=src_ap, op0=AL.logical_shift_left, scalar1=16, scalar2=None)
                    nc.vector.tensor_scalar(out=f1[:], in0=src_ap, op0=AL.bitwise_and, scalar1=_i32(0xFFFF0000), scalar2=None)
                    return f0, f1

                def f32v(t, dims, off=0):
                    return apd(t, off, [list(t.ap[0])] + [list(d) for d in dims]).bitcast(DT.float32)

                # ---- dense levels ----
                def ktree(l, gd, el, ngrp, q, Q, tag):
                    # select k_off window over bits q-1..0; gd lanes
                    # [x*el + g*(Q+1) + t]; returns tile [P, S, ngrp, 2]
                    cur = None
                    for b in range(q - 1, -1, -1):
                        wnew = 2**b + 1 if b > 0 else 2
                        half = 2**b
                        nxt = workp.tile([P, S, ngrp, wnew], DT.int32, tag=f"dt{tag}{b}", name=f"dt{tag}{b}")
                        if cur is None:
                            lo = apd(gd, 0, [list(gd.ap[0]), [el, S], [Q + 1, ngrp], [1, wnew]])
                            hi = apd(gd, half, [list(gd.ap[0]), [el, S], [Q + 1, ngrp], [1, wnew]])
                        else:
                            lo = apd(cur, 0, [list(cur.ap[0]), [cur.ap[1][0], S], [cur.ap[2][0], ngrp], [1, wnew]])
                            hi = apd(cur, half, [list(cur.ap[0]), [cur.ap[1][0], S], [cur.ap[2][0], ngrp], [1, wnew]])
                        mb = workp.tile([P, S], DT.int32, tag=f"dm{b}", name=f"dm{b}")
                        nc.vector.tensor_scalar(out=mb[:], in0=koffs[:, :, l], op0=AL.logical_shift_right, scalar1=b, op1=AL.bitwise_and, scalar2=1)
                        nc.vector.tensor_scalar(out=mb[:], in0=mb[:], op0=AL.mult, scalar1=-1, scalar2=None)
                        mbb = apd(mb, 0, [list(mb.ap[0]), [1, S], [0, ngrp], [0, wnew]])
                        nc.vector.tensor_tensor(out=nxt[:], in0=lo, in1=hi, op=AL.bitwise_xor)
                        nc.vector.tensor_tensor(out=nxt[:], in0=nxt[:], in1=mbb, op=AL.bitwise_and)
                        nc.vector.tensor_tensor(out=nxt[:], in0=nxt[:], in1=lo, op=AL.bitwise_xor)
                        cur = nxt
                    return cur

                for l in DENSE_LV:
                    el = ELEM[l]
                    Q = LEVEL_Q[l]
                    q = Q.bit_length() - 1
                    gd = gdd[l % 2]
                    if l == 7:
                        baseA = tabx[SLOT_OFF[7] : SLOT_OFF[7] + N_SLOTS[7], 0:el]
                        baseB = tabx[SLOT_OFF7B : SLOT_OFF7B + N_SLOTS[7], 0:el]
                        gdA, gdB = gdd[0], gdd[1]
                        for gdX, baseX in ((gdA, baseA), (gdB, baseB)):
                            gview = apd(gdX, 0, [list(gdX.ap[0]), [el, S], [1, el]])
                            dma_gather_raw(
                                nc.gpsimd, gview, baseX,
                                wrp[:, l * 8 * S : (l + 1) * 8 * S], S * P, el,
                            )
                        curA = ktree(l, gdA, el, 2, q, Q, "2a")
                        curB = ktree(l, gdB, el, 2, q, Q, "2b")
                        comb = workp.tile([P, S, 8], DT.int32, tag="comb7", name="comb7")
                        nc.vector.tensor_copy(
                            out=apd(comb, 0, [list(comb.ap[0]), [8, S], [1, 4]]),
                            in_=apd(curA, 0, [list(curA.ap[0]), [4, S], [1, 4]]),
                        )
                        nc.vector.tensor_copy(
                            out=apd(comb, 4, [list(comb.ap[0]), [8, S], [1, 4]]),
                            in_=apd(curB, 0, [list(curB.ap[0]), [4, S], [1, 4]]),
                        )
                        csrc = apd(comb, 0, [list(comb.ap[0]), [1, 8 * S]])
                    else:
                        base = tabx[SLOT_OFF[l] : SLOT_OFF[l] + N_SLOTS[l], 0:el]
                        gview = apd(gd, 0, [list(gd.ap[0]), [el, S], [1, el]])
                        dma_gather_raw(
                            nc.gpsimd, gview, base,
                            wrp[:, l * 8 * S : (l + 1) * 8 * S], S * P, el,
                        )
                        if Q == 1:
                            csrc = apd(gd, 0, [list(gd.ap[0]), [1, 8 * S]])
                        else:
                            cur = ktree(l, gd, el, 4, q, Q, "4g")
                            csrc = apd(cur, 0, [list(cur.ap[0]), [1, 8 * S]])
                    # corners in x-major layout: lane x*8 + m
                    e0, e1 = unpack(csrc, 8 * S, "d")
                    wx = wof(l, 0, 4, minor=True)
                    xt0 = workp.tile([P, S, 4], DT.float32, tag="xt0", name="xt0")
                    xt1 = workp.tile([P, S, 4], DT.float32, tag="xt1", name="xt1")
                    lerp(xt0[:], f32v(e0, [[8, S], [1, 4]]), f32v(e0, [[8, S], [1, 4]], 4), wx)
                    lerp(xt1[:], f32v(e1, [[8, S], [1, 4]]), f32v(e1, [[8, S], [1, 4]], 4), wx)
                    wy = wof(l, 1, 2, minor=True)
                    yt0 = workp.tile([P, S, 2], DT.float32, tag="yt0", name="yt0")
                    yt1 = workp.tile([P, S, 2], DT.float32, tag="yt1", name="yt1")
                    lerp(yt0[:], apd(xt0, 0, [list(xt0.ap[0]), [4, S], [1, 2]]), apd(xt0, 2, [list(xt0.ap[0]), [4, S], [1, 2]]), wy)
                    lerp(yt1[:], apd(xt1, 0, [list(xt1.ap[0]), [4, S], [1, 2]]), apd(xt1, 2, [list(xt1.ap[0]), [4, S], [1, 2]]), wy)
                    wz = wof(l, 2, 1, minor=True)
                    o0 = apd(osb, 2 * l, [list(osb.ap[0]), [32, S]])
                    o1 = apd(osb, 2 * l + 1, [list(osb.ap[0]), [32, S]])
                    lerp(o0, apd(yt0, 0, [list(yt0.ap[0]), [2, S]]), apd(yt0, 1, [list(yt0.ap[0]), [2, S]]), wz)
                    lerp(o1, apd(yt1, 0, [list(yt1.ap[0]), [2, S]]), apd(yt1, 1, [list(yt1.ap[0]), [2, S]]), wz)

                # ---- hash levels ----
                for l in HASH_LV:
                    lh = l - 8
                    base = tabx[SLOT_OFF[l] : SLOT_OFF[l] + N_SLOTS[l], 0:16]
                    cls0 = 8 + lh * 8
                    for h2 in range(4):  # 2 classes per 8192-idx instruction
                        gd = gdh[(4 * lh + h2) % 3]
                        gview = apd(gd, 0, [list(gd.ap[0]), [16, 2 * S], [1, 16]])
                        dma_gather_raw(
                            nc.gpsimd, gview, base,
                            wrp[:, (cls0 + 2 * h2) * 8 * S : (cls0 + 2 * h2 + 2) * 8 * S],
                            2 * S * P, 16,
                        )
                        # extraction tree over 16 lanes for these 2 classes
                        hmv = apd(hm, lh * 8 * S + h2 * 2 * S, [list(hm.ap[0]), [1, 2 * S]])
                        cur = None
                        for b in range(3, -1, -1):
                            half = 2**b
                            nxt = workp.tile([P, 2 * S, half], DT.int32, tag=f"ht{b}", name=f"ht{b}")
                            if cur is None:
                                lo = apd(gd, 0, [list(gd.ap[0]), [16, 2 * S], [1, half]])
                                hi = apd(gd, half, [list(gd.ap[0]), [16, 2 * S], [1, half]])
                            else:
                                lo = cur[:, :, 0:half]
                                hi = cur[:, :, half : 2 * half]
                            mb = workp.tile([P, 2 * S], DT.int32, tag=f"hmk{b}", name=f"hmk{b}")
                            nc.vector.tensor_scalar(out=mb[:], in0=hmv, op0=AL.logical_shift_right, scalar1=b, op1=AL.bitwise_and, scalar2=1)
                            nc.vector.tensor_scalar(out=mb[:], in0=mb[:], op0=AL.mult, scalar1=-1, scalar2=None)
                            mbb = apd(mb, 0, [list(mb.ap[0]), [1, 2 * S], [0, half]])
                            nc.vector.tensor_tensor(out=nxt[:], in0=lo, in1=hi, op=AL.bitwise_xor)
                            nc.vector.tensor_tensor(out=nxt[:], in0=nxt[:], in1=mbb, op=AL.bitwise_and)
                            nc.vector.tensor_tensor(out=nxt[:], in0=nxt[:], in1=lo, op=AL.bitwise_xor)
                            cur = nxt
                        nc.vector.tensor_copy(out=hcor[:, h2 * 2 * S : (h2 + 1) * 2 * S], in_=cur[:, :, 0])
                    # hcor: [P, 8S] corner values, class-major (c*S + x)
                    e0, e1 = unpack(hcor[:, :], 8 * S, "h")
                    wx = wof(l, 0, 4, minor=False)
                    x0 = workp.tile([P, 4 * S], DT.float32, tag="hx0", name="hx0")
                    x1 = workp.tile([P, 4 * S], DT.float32, tag="hx1", name="hx1")
                    lerp(x0[:], f32v(e0, [[1, 4 * S]]), f32v(e0, [[1, 4 * S]], 4 * S), wx)
                    lerp(x1[:], f32v(e1, [[1, 4 * S]]), f32v(e1, [[1, 4 * S]], 4 * S), wx)
                    wy = wof(l, 1, 2, minor=False)
                    y0 = workp.tile([P, 2 * S], DT.float32, tag="hy0", name="hy0")
                    y1 = workp.tile([P, 2 * S], DT.float32, tag="hy1", name="hy1")
                    lerp(y0[:], x0[:, 0 : 2 * S], x0[:, 2 * S : 4 * S], wy)
                    lerp(y1[:], x1[:, 0 : 2 * S], x1[:, 2 * S : 4 * S], wy)
                    wz = wof(l, 2, 1, minor=False)
                    o0 = apd(osb, 2 * l, [list(osb.ap[0]), [32, S]])
                    o1 = apd(osb, 2 * l + 1, [list(osb.ap[0]), [32, S]])
                    lerp(o0, y0[:, 0:S], y0[:, S : 2 * S], wz)
                    lerp(o1, y1[:, 0:S], y1[:, S : 2 * S], wz)

                nc.gpsimd.dma_start(out[:, bass.ds(st, S), :], osb[:])
    nc.compile()
    return nc


# ---------------- host side ----------------

def _pack_bf16(t):
    u = np.ascontiguousarray(t.astype(np.float32)).view(np.uint32).astype(np.uint64)
    r = ((u + 0x7FFF + ((u >> 16) & 1)) >> 16).astype(np.uint32)
    return r[:, 0] | (r[:, 1] << np.uint32(16))


def build_tabx(tables):
    pk = _pack_bf16(tables)
    tabx = np.zeros((TOT_SLOTS, 64), dtype=np.uint32)
    for l in DENSE_LV:
        r, Q, M = RES[l], LEVEL_Q[l], LEVEL_M[l]
        o = SLOT_OFF[l]
        rp1 = r + 1
        seg = pk[OFFS[l] : OFFS[l + 1]]
        if Q == 1:
            ii, jj, kk = np.meshgrid(np.arange(r), np.arange(r), np.arange(r), indexing="ij")
            # reference indexes the (r+1)^3 table with strides r^2, r, 1
            n0 = ((ii * r + jj) * r + kk).ravel()
            lane = 0
            for di in (0, 1):
                for dj in (0, 1):
                    for dk in (0, 1):
                        tabx[o : o + r * r * r, lane] = seg[n0 + di * r * r + dj * r + dk]
                        lane += 1
        else:
            ii, jj, mm = np.meshgrid(np.arange(r), np.arange(r), np.arange(M), indexing="ij")
            if l == 7:
                for di in (0, 1):
                    oo = o if di == 0 else SLOT_OFF7B
                    lane = 0
                    for dj in (0, 1):
                        for t in range(Q + 1):
                            kidx = np.minimum(mm * Q + t, r)
                            row = ((ii + di) * r + (jj + dj)) * r + kidx
                            src = np.where(mm * Q + t <= r, seg[row], 0)
                            tabx[oo : oo + r * r * M, lane] = src.ravel()
                            lane += 1
            else:
                lane = 0
                for di in (0, 1):
                    for dj in (0, 1):
                        for t in range(Q + 1):
                            kidx = np.minimum(mm * Q + t, r)
                            row = ((ii + di) * r + (jj + dj)) * r + kidx
                            src = np.where(mm * Q + t <= r, seg[row], 0)
                            tabx[o : o + r * r * M, lane] = src.ravel()
                            lane += 1
    for l in HASH_LV:
        o, ns = SLOT_OFF[l], N_SLOTS[l]
        seg = pk[OFFS[l] : OFFS[l + 1]]
        tabx[o : o + ns, 0:16] = seg.reshape(ns, 16)
    return tabx.view(np.int32)


def build_cvec():
    cv = np.zeros(128, dtype=np.float32)
    for l in range(N_LEVELS):
        grid = np.float32(2.0) / np.float32(RES[l])
        cv[l] = np.float32(1.0) / grid
        cv[16 + l] = np.float32(RES[l] - 1)
        cv[32 + l] = grid
    for l in DENSE_LV:
        r, Q, M = RES[l], LEVEL_Q[l], LEVEL_M[l]
        cv[48 + l] = np.float32(r * M)
        cv[56 + l] = np.float32(M)
        cv[64 + l] = np.float32(1.0 / Q)
        cv[72 + l] = np.float32(Q)
    return cv.reshape(1, 128)


_NC_CACHE = {}
TRACE = False
LAST_NS = None


def _get_nc(slots, S):
    key = (slots, S)
    if key not in _NC_CACHE:
        _NC_CACHE[key] = build_kernel(slots, S)
    return _NC_CACHE[key]


def kernel(x: np.ndarray, tables: np.ndarray) -> np.ndarray:
    global LAST_NS
    from concourse.bass_utils import run_bass_kernel_spmd

    B = x.shape[0]
    per_core = B // N_CORES
    slots = per_core // P
    S = min(32, slots)
    nc = _get_nc(slots, S)
    tabx = build_tabx(tables)
    cv = build_cvec()
    in_maps = []
    for c in range(N_CORES):
        xs = np.ascontiguousarray(
            x[c * per_core : (c + 1) * per_core].reshape(P, slots, 3)
        ).astype(np.float32)
        in_maps.append({"x": xs, "tabx": tabx, "cvec": cv})
    kw = {"trace": True} if TRACE else {}
    res = run_bass_kernel_spmd(nc, in_maps, core_ids=list(range(N_CORES)), **kw)
    LAST_NS = res.exec_time_ns
    outs = [res.results[c]["out"].reshape(per_core, 32) for c in range(N_CORES)]
    return np.concatenate(outs, axis=0).astype(np.float32)
